# revision 10
# baseline (speedup 1.0000x reference)
"""Per-row L2 normalization on 8 Trainium2 NeuronCores.

Full input: tensor [16384, 4096] f32.  out[r, :] = x[r, :] / sqrt(sum(x[r, :]**2))

Sharding: data-parallel on rows — core c gets rows [c*2048, (c+1)*2048).
Each row's reduction is local to its core; no communication.

Per-core kernel (SPMD, identical program on all 8 cores):
  - 8 tiles of 256 rows, laid out [128 partitions x 2 rows x 4096] so each
    partition reads one contiguous 32 KiB chunk of DRAM per tile (4 MiB DMAs).
  - ACT (ScalarE): Square activation with accum_out -> per-row sum of squares
    in a single pass (the squared values go to a scratch tile that is never
    read).  ~62 us/core total.
  - DVE (VectorE): reciprocal of the sum, then per-row scale multiply.
    ~72 us/core total.
  - ACT: Sqrt of the reciprocal -> 1/sqrt(ss); a Newton-Raphson step on DVE
    refines it (ACT Sqrt spline has a loose ULP budget; with the refinement
    max elementwise rel err is 8.6e-6, norm rel err 3.2e-6).
  - Loads issued on SyncE HWDGE, stores on GpSimd SWDGE so the two DMA
    directions flow through separate issue paths; xp bufs=4 keeps several
    DMAs in flight.

This is memory-bound: 32 MiB read + 32 MiB written per core = 64 MiB at the
~358 GB/s per-NC HBM limit -> ~187 us roofline.  Measured steady-state
(repeat-differencing single NEFF dispatches): ~184-192 us/exec; TimelineSim
cost model predicts 190 us.  Compute engines sit at ~35-40%% occupancy, so
the kernel tracks the DMA roofline.
"""

import numpy as np

import concourse.bacc as bacc
import concourse.bass as bass
import concourse.mybir as mybir
import concourse.tile as tile
from concourse.bass_utils import run_bass_kernel_spmd

N_CORES = 8
ROWS = 16384
D = 4096
RPC = ROWS // N_CORES  # rows per core = 2048
P = 128  # SBUF partitions
NR = 2  # rows per partition per tile
TILE_ROWS = P * NR  # 256
NTILES = RPC // TILE_ROWS  # 8

_CACHE: dict[str, bass.Bass] = {}


def _build_nc(
    repeats: int = 1,
    nr: int = NR,
    bufs: int = 4,
    load_eng: str = "sync",
    store_eng: str = "gpsimd",
    sq_on_dve: bool = False,
) -> bass.Bass:
    """Build the per-core Bass program. repeats>1 replays the whole tile loop
    (same input -> same output) for benchmark timing only."""
    nc = bacc.Bacc()
    x = nc.dram_tensor("tensor", [RPC, D], mybir.dt.float32, kind="ExternalInput")
    y = nc.dram_tensor("out", [RPC, D], mybir.dt.float32, kind="ExternalOutput")

    ntiles = RPC // (P * nr)
    # Tile t covers rows [t*P*nr, (t+1)*P*nr); partition p holds nr
    # consecutive rows (contiguous nr*16 KiB per partition).
    xv = x[:, :].rearrange("(t p n) d -> t p n d", p=P, n=nr)
    yv = y[:, :].rearrange("(t p n) d -> t p n d", p=P, n=nr)

    ld = getattr(nc, load_eng)
    st = getattr(nc, store_eng)

    f32 = mybir.dt.float32
    with tile.TileContext(nc) as tc:
        with (
            tc.tile_pool(name="xp", bufs=bufs) as xp,
            tc.tile_pool(name="sq", bufs=2) as sqp,
            tc.tile_pool(name="st", bufs=8) as stp,
        ):
            for t in [t for _ in range(repeats) for t in range(ntiles)]:
                xt = xp.tile([P, nr, D], f32)
                ld.dma_start(out=xt[:, :, :], in_=xv[t])

                ss = stp.tile([P, nr], f32)
                for j in range(nr):
                    sq = sqp.tile([P, D], f32, tag="sq")
                    if sq_on_dve:
                        nc.vector.tensor_tensor_reduce(
                            out=sq[:, :],
                            in0=xt[:, j, :],
                            in1=xt[:, j, :],
                            scale=1.0,
                            scalar=0.0,
                            op0=mybir.AluOpType.mult,
                            op1=mybir.AluOpType.add,
                            accum_out=ss[:, j : j + 1],
                        )
                    else:
                        nc.scalar.activation(
                            out=sq[:, :],
                            in_=xt[:, j, :],
                            func=mybir.ActivationFunctionType.Square,
                            accum_out=ss[:, j : j + 1],
                        )

                inv = stp.tile([P, nr], f32)
                nc.vector.reciprocal(out=inv[:, :], in_=ss[:, :])
                rn = stp.tile([P, nr], f32)
                nc.scalar.activation(
                    out=rn[:, :],
                    in_=inv[:, :],
                    func=mybir.ActivationFunctionType.Sqrt,
                )
                # Newton-Raphson: y' = y*(1.5 - 0.5*ss*y^2) cleans up the ACT
                # Sqrt approximation to full fp32 accuracy.
                t0 = stp.tile([P, nr], f32)
                nc.vector.tensor_mul(out=t0[:, :], in0=rn[:, :], in1=rn[:, :])
                nc.vector.tensor_mul(out=t0[:, :], in0=t0[:, :], in1=ss[:, :])
                nc.vector.tensor_scalar_mul(out=t0[:, :], in0=t0[:, :], scalar1=-0.5)
                nc.vector.tensor_scalar_add(out=t0[:, :], in0=t0[:, :], scalar1=1.5)
                nc.vector.tensor_mul(out=rn[:, :], in0=rn[:, :], in1=t0[:, :])

                for j in range(nr):
                    nc.vector.tensor_scalar_mul(
                        out=xt[:, j, :],
                        in0=xt[:, j, :],
                        scalar1=rn[:, j : j + 1],
                    )
                st.dma_start(out=yv[t], in_=xt[:, :, :])
    nc.finalize()
    return nc


def kernel(tensor: np.ndarray) -> np.ndarray:
    x = np.ascontiguousarray(np.asarray(tensor, dtype=np.float32))
    assert x.shape == (ROWS, D), x.shape

    if "nc" not in _CACHE:
        _CACHE["nc"] = _build_nc()
    nc = _CACHE["nc"]

    in_maps = [
        {"tensor": np.ascontiguousarray(x[c * RPC : (c + 1) * RPC])}
        for c in range(N_CORES)
    ]
    res = run_bass_kernel_spmd(nc, in_maps, core_ids=list(range(N_CORES)))
    return np.concatenate([res.results[c]["out"] for c in range(N_CORES)], axis=0)


# revision 12
# speedup vs baseline: 1.1094x; 1.1094x over previous
"""Per-row L2 normalization on 8 Trainium2 NeuronCores.

Full input: tensor [16384, 4096] f32.  out[r, :] = x[r, :] / sqrt(sum(x[r, :]**2))

Sharding: data-parallel on rows — core c gets rows [c*2048, (c+1)*2048).
Each row's reduction is local to its core; no communication.

Per-core kernel (SPMD, identical program on all 8 cores):
  - 8 tiles of 256 rows, laid out [128 partitions x 2 rows x 4096] so each
    partition reads one contiguous 32 KiB chunk of DRAM per tile (4 MiB DMAs).
  - ACT (ScalarE): Square activation with accum_out -> per-row sum of squares
    in a single pass (the squared values go to a scratch tile that is never
    read).  ~62 us/core total.
  - DVE (VectorE): reciprocal of the sum, then per-row scale multiply.
    ~72 us/core total.
  - ACT: Sqrt of the reciprocal -> 1/sqrt(ss); a Newton-Raphson step on DVE
    refines it (ACT Sqrt spline has a loose ULP budget; with the refinement
    max elementwise rel err is 8.6e-6, norm rel err 3.2e-6).
  - Loads issued on SyncE HWDGE, stores on GpSimd SWDGE so the two DMA
    directions flow through separate issue paths; xp bufs=4 keeps several
    DMAs in flight.

This is memory-bound: 32 MiB read + 32 MiB written per core = 64 MiB at the
~358 GB/s per-NC HBM limit -> ~187 us roofline.  Measured steady-state
(repeat-differencing single NEFF dispatches): ~184-200 us/exec; TimelineSim
cost model predicts 190 us.  Compute engines sit at ~35-40% occupancy, so
the kernel tracks the DMA roofline.
"""

import numpy as np

import concourse.bacc as bacc
import concourse.bass as bass
import concourse.mybir as mybir
import concourse.tile as tile
from concourse.bass_utils import run_bass_kernel_spmd

N_CORES = 8
ROWS = 16384
D = 4096
RPC = ROWS // N_CORES  # rows per core = 2048
P = 128  # SBUF partitions
NR = 2  # rows per partition per tile
TILE_ROWS = P * NR  # 256
NTILES = RPC // TILE_ROWS  # 8

_CACHE: dict[str, bass.Bass] = {}


def _build_nc(
    repeats: int = 1,
    nr: int = NR,
    bufs: int = 4,
    load_eng: str = "sync",
    store_eng: str = "gpsimd",
    sq_on_dve: bool = False,
) -> bass.Bass:
    """Build the per-core Bass program. repeats>1 replays the whole tile loop
    (same input -> same output) for benchmark timing only."""
    nc = bacc.Bacc()
    x = nc.dram_tensor("tensor", [RPC, D], mybir.dt.float32, kind="ExternalInput")
    y = nc.dram_tensor("out", [RPC, D], mybir.dt.float32, kind="ExternalOutput")

    ntiles = RPC // (P * nr)
    # Tile t covers rows [t*P*nr, (t+1)*P*nr); partition p holds nr
    # consecutive rows (contiguous nr*16 KiB per partition).
    xv = x[:, :].rearrange("(t p n) d -> t p n d", p=P, n=nr)
    yv = y[:, :].rearrange("(t p n) d -> t p n d", p=P, n=nr)

    ld = getattr(nc, load_eng)
    st = getattr(nc, store_eng)

    f32 = mybir.dt.float32
    with tile.TileContext(nc) as tc:
        with (
            tc.tile_pool(name="xp", bufs=bufs) as xp,
            tc.tile_pool(name="sq", bufs=2) as sqp,
            tc.tile_pool(name="st", bufs=8) as stp,
        ):
            for t in [t for _ in range(repeats) for t in range(ntiles)]:
                xt = xp.tile([P, nr, D], f32)
                ld.dma_start(out=xt[:, :, :], in_=xv[t])

                ss = stp.tile([P, nr], f32)
                for j in range(nr):
                    sq = sqp.tile([P, D], f32, tag="sq")
                    if sq_on_dve:
                        nc.vector.tensor_tensor_reduce(
                            out=sq[:, :],
                            in0=xt[:, j, :],
                            in1=xt[:, j, :],
                            scale=1.0,
                            scalar=0.0,
                            op0=mybir.AluOpType.mult,
                            op1=mybir.AluOpType.add,
                            accum_out=ss[:, j : j + 1],
                        )
                    else:
                        nc.scalar.activation(
                            out=sq[:, :],
                            in_=xt[:, j, :],
                            func=mybir.ActivationFunctionType.Square,
                            accum_out=ss[:, j : j + 1],
                        )

                inv = stp.tile([P, nr], f32)
                nc.vector.reciprocal(out=inv[:, :], in_=ss[:, :])
                rn = stp.tile([P, nr], f32)
                nc.scalar.activation(
                    out=rn[:, :],
                    in_=inv[:, :],
                    func=mybir.ActivationFunctionType.Sqrt,
                )
                # Newton-Raphson: y' = y*(1.5 - 0.5*ss*y^2) cleans up the ACT
                # Sqrt approximation to full fp32 accuracy.
                t0 = stp.tile([P, nr], f32)
                nc.vector.tensor_mul(out=t0[:, :], in0=rn[:, :], in1=rn[:, :])
                nc.vector.tensor_mul(out=t0[:, :], in0=t0[:, :], in1=ss[:, :])
                nc.vector.tensor_scalar_mul(out=t0[:, :], in0=t0[:, :], scalar1=-0.5)
                nc.vector.tensor_scalar_add(out=t0[:, :], in0=t0[:, :], scalar1=1.5)
                nc.vector.tensor_mul(out=rn[:, :], in0=rn[:, :], in1=t0[:, :])

                for j in range(nr):
                    nc.vector.tensor_scalar_mul(
                        out=xt[:, j, :],
                        in0=xt[:, j, :],
                        scalar1=rn[:, j : j + 1],
                    )
                st.dma_start(out=yv[t], in_=xt[:, :, :])
    nc.finalize()
    return nc


def kernel(tensor: np.ndarray) -> np.ndarray:
    x = np.ascontiguousarray(np.asarray(tensor, dtype=np.float32))
    assert x.shape == (ROWS, D), x.shape

    if "nc" not in _CACHE:
        _CACHE["nc"] = _build_nc()
    nc = _CACHE["nc"]

    in_maps = [
        {"tensor": np.ascontiguousarray(x[c * RPC : (c + 1) * RPC])}
        for c in range(N_CORES)
    ]
    res = run_bass_kernel_spmd(nc, in_maps, core_ids=list(range(N_CORES)))
    return np.concatenate([res.results[c]["out"] for c in range(N_CORES)], axis=0)
